# revision 1
# baseline (speedup 1.0000x reference)
"""Trainium2 Bass kernel for nn_ClusteringLayer (vq_codebook).

Math (ALPHA=1.0, so the power term is a no-op):
    dist[n,k] = ||x_n||^2 + ||c_k||^2 - 2 x_n.c_k
    s[n,k]    = 1 + dist[n,k]
    q[n,k]    = (1/s[n,k]) / sum_k (1/s[n,k])

Strategy (data-parallel over N across 8 cores, clusters replicated):
  Host prep (layout only + tiny affine on the small cluster table):
    - xt  = x_shard.T               [D=512, NS=8192]  (contraction dim on partitions)
    - cm  = -2 * clusters.T         [512, 256]
    - ct  = [ones; 1 + ||c||^2]     [2, 256]
  Device (per 512-row block, 16 blocks):
    - DMA xt block -> SBUF [128, 4, 512] (d-chunk major)
    - ACT: square -> xtsq; PE: ones.T @ xtsq -> x2 row [1, 512] (per-row norms)
    - per 128-row chunk: psum = sum_j xt_j.T @ cm_j + [x2; 1].T @ ct = s
      (fp32r matmuls: full-rate fp32 path on the PE at N=256)
    - DVE: qun = 1/psum; rowsum; inv = 1/rowsum
    - GPSIMD: q = qun * inv ; DMA out

Scheduling discipline: the walrus codegen in this stack allows AT MOST ONE
semaphore wait per compute-engine instruction (PE/ACT/DVE; SP DMAs take
several).  Tile emits a wait per cross-engine dependency AND per
same-address WAW (posted writes need the previous write's ack).  So every
instruction here is arranged to need at most one new semaphore, using tiny
"absorber" ops (striped over never-reused trash addresses so they create
no WAW of their own) that pre-observe extra semaphores on the same engine.
"""

import numpy as np
import concourse.bass as bass
import concourse.tile as tile
from concourse import mybir
from concourse.bass_utils import run_bass_kernel_spmd

N, K, D = 65536, 256, 512
NCORES = 8
NS = N // NCORES            # 8192 rows per core
BLK = 512                   # rows per block
NBLK = NS // BLK            # 16 blocks
NCH = BLK // 128            # 4 chunks of 128 rows per block
DCH = D // 128              # 4 d-chunks
XT_BUFS = 2
XTSQ_BUFS = 2
QUN_BUFS = 2
OUT_BUFS = 2
PS_BUFS = 4
X2_BUFS = 2

F32R = mybir.dt.float32r
F32 = mybir.dt.float32
COPY = mybir.ActivationFunctionType.Copy
SQUARE = mybir.ActivationFunctionType.Square


def build_nc():
    nc = bass.Bass()
    xt = nc.declare_dram_parameter("xt", [D, NS], F32R, isOutput=False)
    cm = nc.declare_dram_parameter("cm", [D, K], F32R, isOutput=False)
    ct = nc.declare_dram_parameter("ct", [2, K], F32R, isOutput=False)
    q = nc.declare_dram_parameter("q", [NS, K], F32, isOutput=True)

    # DRAM views: d-chunk-major for x, row-chunk-major for q
    xt_v = xt[:, :].rearrange("(j p) n -> p j n", p=128)      # [128, 4, 8192]
    cm_v = cm[:, :].rearrange("(j p) k -> p j k", p=128)      # [128, 4, 256]
    q_v = q[:, :].rearrange("(r p) k -> p r k", p=128)        # [128, 64, 256]

    with tile.TileContext(nc) as tc:
        with (
            tc.tile_pool(name="consts", bufs=1) as constp,
            tc.tile_pool(name="xt", bufs=XT_BUFS) as xtp,
            tc.tile_pool(name="xtsq", bufs=XTSQ_BUFS) as xtsqp,
            tc.tile_pool(name="aug", bufs=1) as augp,
            tc.tile_pool(name="qun", bufs=QUN_BUFS) as qunp,
            tc.tile_pool(name="small", bufs=4) as smallp,
            tc.tile_pool(name="outs", bufs=OUT_BUFS) as outp,
            tc.tile_pool(name="psm", bufs=PS_BUFS, space="PSUM") as psp,
            tc.tile_pool(name="psx2", bufs=X2_BUFS, space="PSUM") as psx2p,
        ):
            # ---- prologue: constants + cluster table ----
            c_sb = constp.tile([128, DCH, K], F32R)
            cdma = nc.sync.dma_start(out=c_sb, in_=cm_v)
            ct_sb = constp.tile([2, K], F32R)
            ctdma = nc.sync.dma_start(out=ct_sb, in_=ct[:, :])
            # fp32 staging memsets (DVE memset cannot encode fp32r); the ACT
            # copies below round them into the fp32r tiles the PE reads
            ones_col_f = constp.tile([128, 1], F32)
            nc.vector.memset(ones_col_f, 1.0)
            ones1_f = constp.tile([1, 1], F32)
            nc.vector.memset(ones1_f, 1.0)
            aug_f = constp.tile([2, BLK], F32)
            nc.vector.memset(aug_f, 1.0)
            ones_col = constp.tile([128, 1], F32R)
            cp1 = nc.scalar.activation(out=ones_col, in_=ones_col_f, func=COPY)
            ones1 = constp.tile([1, 1], F32R)
            cp2 = nc.scalar.activation(out=ones1, in_=ones1_f, func=COPY)
            aug_sb = augp.tile([2, BLK], F32R)
            # row 1 is the "ones" contraction row; row 0 is rewritten per block
            cp3 = nc.scalar.activation(out=aug_sb[0:2, :], in_=aug_f, func=COPY)

            # striped trash (each absorber writes its own never-reused column)
            act_trash = constp.tile([1, 96], F32R)     # ACT absorber targets
            dve_trash = constp.tile([1, 192], F32)     # DVE absorber targets
            pool_trash = constp.tile([1, 64], F32)     # Pool absorber targets
            act_t = iter(range(96))
            dve_t = iter(range(192))
            pool_t = iter(range(64))

            def pe_absorb(dep, nosync_after=None, read=None):
                # standalone bf16 ldweights: a real PE instruction that can
                # carry one semaphore wait but writes no memory (the real
                # fp32r matmuls all self-load their weights afterwards)
                src = (ones1 if read is None else read).bitcast(mybir.dt.bfloat16)
                a = nc.tensor.ldweights(weights=src)
                if dep is not None:
                    tile.add_dep_helper(a.ins, dep.ins, sync=True,
                                        reason="pe absorb")
                if nosync_after is not None:
                    tile.add_dep_helper(a.ins, nosync_after.ins, sync=False,
                                        reason="pe order")
                return a

            def act_absorb(dep, nosync_after=None, read=None):
                i = next(act_t)
                a = nc.scalar.activation(
                    out=act_trash[0:1, i:i + 1],
                    in_=ones1 if read is None else read, func=COPY)
                if dep is not None:
                    tile.add_dep_helper(a.ins, dep.ins, sync=True,
                                        reason="act absorb")
                if nosync_after is not None:
                    tile.add_dep_helper(a.ins, nosync_after.ins, sync=False,
                                        reason="act order")
                return a

            def dve_absorb(dep, nosync_after=None, read=None):
                i = next(dve_t)
                a = nc.vector.tensor_copy(
                    out=dve_trash[0:1, i:i + 1],
                    in_=dve_trash[0:1, i:i + 1] if read is None else read)
                if dep is not None:
                    tile.add_dep_helper(a.ins, dep.ins, sync=True,
                                        reason="dve absorb")
                if nosync_after is not None:
                    tile.add_dep_helper(a.ins, nosync_after.ins, sync=False,
                                        reason="dve order")
                return a

            def pool_absorb(dep, nosync_after=None):
                i = next(pool_t)
                a = nc.gpsimd.memset(pool_trash[0:1, i:i + 1], 0.0)
                if dep is not None:
                    tile.add_dep_helper(a.ins, dep.ins, sync=True,
                                        reason="pool absorb")
                if nosync_after is not None:
                    tile.add_dep_helper(a.ins, nosync_after.ins, sync=False,
                                        reason="pool order")
                return a

            def order(a, b):
                tile.add_dep_helper(a.ins, b.ins, sync=False, reason="order")

            # boot chain: PE observes the ACT const-copies + the two const DMAs
            pb1 = pe_absorb(cp3)                       # waits ACT (last copy)
            pb2 = pe_absorb(None, nosync_after=pb1, read=c_sb[0:1, 0, 0:1])
            pb3 = pe_absorb(None, nosync_after=pb2, read=ct_sb[0:1, 0:1])
            ab0 = cp3                                  # ACT already ordered

            # histories for cross-iteration hazard absorption
            sq_h, x2cp_h, x2mm_h, aug_h = {}, {}, {}, {}
            recip_h, mul_h, outdma_h, mmlast_h, xdma_h = {}, {}, {}, {}, {}
            prev_pe = pb3
            prev_act = ab0

            for b in range(NBLK):
                # ACT absorber so the ACT-issued DMA below needs no PE wait
                # (xtile slot WAR vs the b-2 matmuls)
                aa0 = act_absorb(mmlast_h.get(b - XT_BUFS), nosync_after=prev_act)
                xtile = xtp.tile([128, DCH, BLK], F32R)
                # issued from the ACT engine so the ACT-reader WAR (square of
                # b-2) is covered by ACT program order, not a semaphore
                xdma = nc.scalar.dma_start(
                    out=xtile, in_=xt_v[:, :, b * BLK:(b + 1) * BLK])
                order(xdma, aa0)

                # ACT: observe xt-DMA completion; then own xtsq WAW (square b-2)
                aa1 = act_absorb(None, nosync_after=None,
                                 read=xtile[0:1, 0, 0:1])
                aa2 = act_absorb(sq_h.get(b - XTSQ_BUFS), nosync_after=aa1)
                xtsq = xtsqp.tile([128, DCH, BLK], F32R)
                sq = nc.scalar.activation(out=xtsq, in_=xtile, func=SQUARE)
                order(sq, aa2)
                sq_h[b] = sq

                # PE: observe xt-DMA; then observe x2ps WAW (x2mm of b-2)
                pa1 = pe_absorb(None, nosync_after=prev_pe,
                                read=xtile[0:1, 0, 0:1])
                pa2 = pe_absorb(x2mm_h.get(b - X2_BUFS), nosync_after=pa1)

                # x2 row: ones.T @ xtsq accumulated over d-chunks -> [1, BLK]
                x2ps = psx2p.tile([1, BLK], F32)
                x2mm = None
                for j in range(DCH):
                    x2mm = nc.tensor.matmul(
                        x2ps, ones_col[:, 0:1], xtsq[:, j, :],
                        start=(j == 0), stop=(j == DCH - 1))
                    if j == 0:
                        order(x2mm, pa2)
                x2mm_h[b] = x2mm

                # ACT: observe aug row0 WAW (x2cp of b-1), then write x2 row
                aa3 = act_absorb(x2cp_h.get(b - 1, cp3), nosync_after=sq)
                x2cp = nc.scalar.activation(
                    out=aug_sb[0:1, :], in_=x2ps[0:1, :], func=COPY)
                order(x2cp, aa3)
                x2cp_h[b] = x2cp

                # Pool: observe out-tile WAR (out-DMA b-2) + WAW (muls b-2)
                out_tile = outp.tile([128, NCH, K], F32)
                la1 = pool_absorb(outdma_h.get(b - OUT_BUFS))
                la2 = pool_absorb(mul_h.get((b - OUT_BUFS, NCH - 1)), nosync_after=la1)
                prev_pool = la2

                pe_prev_in_block = x2mm
                for c in range(NCH):
                    gc = b * NCH + c
                    # PE: observe ps WAR (recip b-1, same chunk slot)
                    pa3 = pe_absorb(recip_h.get(gc - PS_BUFS),
                                    nosync_after=pe_prev_in_block)
                    ps = psp.tile([128, K], F32)
                    mm0 = nc.tensor.matmul(
                        ps, xtile[:, 0, c * 128:(c + 1) * 128], c_sb[:, 0, :],
                        start=True, stop=False)
                    order(mm0, pa3)
                    for j in range(1, DCH):
                        mmj = nc.tensor.matmul(
                            ps, xtile[:, j, c * 128:(c + 1) * 128], c_sb[:, j, :],
                            start=False, stop=False)
                    if c == NCH - 1:
                        mmlast_h[b] = mmj
                    aug = nc.tensor.matmul(
                        ps, aug_sb[0:2, c * 128:(c + 1) * 128], ct_sb[0:2, :],
                        start=False, stop=True)
                    aug_h[gc] = aug
                    pe_prev_in_block = aug

                    # DVE: observe psum-ready (PE) + qun WAR (Pool mul gc-2)
                    da1 = dve_absorb(None, read=ps[0:1, 0:1])
                    gq = gc - QUN_BUFS
                    da2 = dve_absorb(mul_h.get((gq // NCH, gq % NCH)),
                                     nosync_after=da1,
                                     read=pool_trash[0:1, 0:1])
                    qun = qunp.tile([128, K], F32)
                    rec = nc.vector.reciprocal(qun, ps)
                    order(rec, da2)
                    recip_h[gc] = rec
                    rs = smallp.tile([128, 1], F32)
                    nc.vector.reduce_sum(rs, qun, axis=mybir.AxisListType.X)
                    inv = smallp.tile([128, 1], F32)
                    inv_i = nc.vector.reciprocal(inv, rs)

                    mul = nc.gpsimd.tensor_scalar_mul(
                        out=out_tile[:, c, :], in0=qun, scalar1=inv)
                    order(mul, prev_pool)
                    mul_h[(b, c)] = mul
                    prev_pool = mul

                # SWDGE (Pool-issued) so data-ready is Pool program order
                od = nc.gpsimd.dma_start(
                    out=q_v[:, b * NCH:(b + 1) * NCH, :], in_=out_tile)
                order(od, prev_pool)
                outdma_h[b] = od
                xdma_h[b] = xdma
                prev_pe = pe_prev_in_block
                prev_act = x2cp
                last_dve = inv_i

            # SP epilogue: pre-observe every outstanding semaphore one drain
            # at a time, so Tile's kernel-tail drain needs no waits of its own
            # (it too can carry only a limited number of sync commands).
            tail_deps = [cdma, ctdma, prev_pe, last_dve, prev_act, mul_h[(NBLK - 1, NCH - 1)]]
            tail_deps += [xdma_h[b] for b in range(max(0, NBLK - 8), NBLK)]
            tail_deps += [outdma_h[b] for b in range(max(0, NBLK - 8), NBLK)]
            prev_drain = None
            for dep in tail_deps:
                dr = nc.sync.drain()
                tile.add_dep_helper(dr.ins, dep.ins, sync=True,
                                    reason="tail pre-observe")
                if prev_drain is not None:
                    order(dr, prev_drain)
                prev_drain = dr
    return nc


def _host_prep(inputs, clusters):
    inputs = np.ascontiguousarray(np.asarray(inputs, dtype=np.float32))
    clusters = np.ascontiguousarray(np.asarray(clusters, dtype=np.float32))
    cm = np.ascontiguousarray(-2.0 * clusters.T)              # [512, 256]
    c2 = np.einsum("kd,kd->k", clusters, clusters)            # [256]
    ct = np.stack([np.ones_like(c2), 1.0 + c2]).astype(np.float32)  # [2, 256]
    in_maps = []
    for i in range(NCORES):
        shard = inputs[i * NS:(i + 1) * NS]                   # [8192, 512]
        xt = np.ascontiguousarray(shard.T)                    # [512, 8192]
        in_maps.append({"xt": xt, "cm": cm, "ct": ct})
    return in_maps


def kernel(inputs, clusters):
    nc = build_nc()
    in_maps = _host_prep(inputs, clusters)
    res = run_bass_kernel_spmd(nc, in_maps, core_ids=list(range(NCORES)))
    out = np.concatenate([res.results[i]["q"] for i in range(NCORES)], axis=0)
    return out.astype(np.float32)

